# revision 1
# baseline (speedup 1.0000x reference)
"""Contrastive (Cauchy-kernel InfoNCE) loss on 8 Trainium2 NeuronCores.

Math: for anchors a_i = features[i] (i < b) and the canonical full-batch
neighbor indices (pos = i+b in column 0, negatives = everything except
self in both halves), the loss reduces to

    D[i, n]  = ||a_i||^2 + ||f_n||^2 - 2 a_i . f_n          (squared dist)
    P[i, n]  = 1 / (1 + D[i, n])                            (Cauchy probit)
    S_i      = sum_n P[i, n] - P[i, i]                      (all but self)
    loss     = mean_i [ log(S_i) - log(P[i, i+b]) ]

so the whole gather collapses into one [128 x 128] x [128 x 2048] matmul
per core plus row reductions and two diagonal extractions.

Sharding: data-parallel over anchors. Core c owns anchors c*128..(c+1)*128.
Each core receives the full feature matrix, pre-transposed ([dim, 2b]) and
block-permuted so that its own anchor block is columns 0:128 and its
positive block is columns 128:256 (making the diagonal extractions the
same static pattern on every core). Each core emits the partial loss sum
of its 128 anchors; the host sums 8 scalars and divides by b.
"""

import numpy as np
import orjson

import concourse.bass as bass
import concourse.bass2jax as bass2jax
import concourse.bass_utils as bass_utils
import concourse.mybir as mybir
import concourse.tile as tile
from concourse.masks import make_identity
from concourse.bass_utils import run_bass_kernel_spmd

B = 1024
DIM = 128
N = 2 * B            # 2048 feature rows
NCORES = 8
PB = B // NCORES     # 128 anchors per core
CH = 512             # free-dim chunk (one PSUM bank)
NCH = N // CH        # 4
F32 = mybir.dt.float32

# Set by a driver to profile the HW execution (requires an NTFF hook).
TRACE = False
LAST_RESULT = None


def _split_multi_waits(bir_json: bytes) -> bytes:
    """The walrus build here accepts only ONE sync-wait per instruction,
    while Tile freely attaches several (one per producer proc). Engines pop
    their queues in order, so hoisting the extra waits onto injected NoOps
    immediately before the instruction is semantically identical."""
    m = orjson.loads(bir_json)
    changed = False
    for fn in m.get("functions", []):
        for blk in fn.get("blocks", []):
            out = []
            for inst in blk.get("instructions", []):
                si = inst.get("sync_info")
                ow = (si or {}).get("on_wait") or []
                if len(ow) > 1:
                    changed = True
                    for k, w in enumerate(ow[:-1]):
                        out.append(
                            {
                                "debug": inst.get("debug", 0),
                                "engine": inst["engine"],
                                "ins": [],
                                "outs": [],
                                "name": f"{inst['name']}-w{k}",
                                "opcode": "NoOp",
                                "text_hint": "wait_split",
                                "sync_info": {"on_update": [], "on_wait": [w]},
                            }
                        )
                    si["on_wait"] = [ow[-1]]
                if inst.get("op_name") == "EVENT_SEMAPHORE_RANGE_CLEAR":
                    inst["engine"] = "SP"
                    changed = True
                out.append(inst)
            blk["instructions"] = out
    return orjson.dumps(m) if changed else bir_json


def _patch_compiler():
    if getattr(bass_utils, "_wait_split_patch", False):
        return
    orig = bass_utils.compile_bir_kernel

    def patched(bir_json, tmpdir, neff_name="file.neff"):
        return orig(_split_multi_waits(bir_json), tmpdir, neff_name=neff_name)

    bass_utils.compile_bir_kernel = patched
    bass2jax.compile_bir_kernel = patched
    bass_utils._wait_split_patch = True


def _build_canonical():
    """Per-core program: ftp [DIM, N] (transposed, block-permuted features)
    -> out [1, 1] partial loss sum over this core's 128 anchors."""
    _patch_compiler()
    nc = bass.Bass(enable_partition_id=False)
    ftp = nc.dram_tensor("ftp", [DIM, N], mybir.dt.float32r, kind="ExternalInput")
    anch = nc.dram_tensor("anch", [128, DIM], F32, kind="ExternalInput")
    out = nc.dram_tensor("out", [1, 1], F32, kind="ExternalOutput")
    BF16 = mybir.dt.bfloat16

    with tile.TileContext(nc) as tc:
        with (
            tc.tile_pool(name="consts", bufs=1) as consts,
            tc.tile_pool(name="big", bufs=1) as big,
            tc.tile_pool(name="small", bufs=1) as small,
            tc.tile_pool(name="psum", bufs=1, space="PSUM") as psum,
        ):
            ones_col = consts.tile([128, 1], F32, tag="ones_col")
            nc.vector.memset(ones_col, 1.0)
            ones128 = consts.tile([128, 128], BF16, tag="ones128")
            nc.vector.memset(ones128, 1.0)
            ident = consts.tile([128, 128], F32, tag="ident")
            make_identity(nc, ident)

            ft = big.tile([128, N], mybir.dt.float32r, tag="ft")
            # bf16 is enough for the sq-norm broadcast matmul (errors are
            # random across the 2047-term row sum) and it runs single-pass
            # on the PE, unlike fp32's LOW_HIGH two-pass mode.
            ft2 = big.tile([128, N], BF16, tag="ft2")
            prob = big.tile([128, N], F32, tag="prob")

            # Warm the ACT Ln/Exp table early so the ~2.7us table load
            # overlaps with DMA/matmul instead of the critical tail.
            logwarm = small.tile([1, 1], F32, tag="logwarm")
            nc.scalar.activation(
                logwarm, ones_col[0:1, 0:1], mybir.ActivationFunctionType.Ln
            )

            anch_sb = small.tile([128, DIM], F32, tag="anch_sb")
            nc.scalar.dma_start(out=anch_sb[:], in_=anch[:])
            dma_engs = [nc.sync, nc.scalar, nc.sync, nc.scalar]
            for j in range(NCH):
                sl = slice(j * CH, (j + 1) * CH)
                dma_engs[j].dma_start(out=ft[:, sl], in_=ftp[:, sl])

            # asq1_col[i] = 1 + ||a_i||^2 via ACT square + free row-accum
            scr_a = small.tile([128, DIM], F32, tag="scr_a")
            asq0 = small.tile([128, 1], F32, tag="asq0")
            nc.scalar.activation(
                scr_a,
                anch_sb,
                mybir.ActivationFunctionType.Square,
                accum_out=asq0,
            )
            asq1_col = small.tile([128, 1], F32, tag="asq1_col")
            nc.scalar.add(asq1_col, asq0, 1.0)

            # anchors^T * -2 (anchors are fperm block 0 => ftp columns 0:128)
            atm2 = small.tile([128, 128], mybir.dt.float32r, tag="atm2")
            nc.vector.tensor_scalar_mul(atm2, ft[:, 0:128], -2.0)

            # ft2 = ft * ft (bf16 out) on DVE, keeping ACT free for Ln/Exp
            for j in range(NCH):
                sl = slice(j * CH, (j + 1) * CH)
                nc.vector.tensor_mul(ft2[:, sl], ft[:, sl].bitcast(F32), ft[:, sl].bitcast(F32))

            # Dummy matmuls while DMA streams in: keeps the PE busy past
            # the HAM activity window so the real matmuls run at 2.4 GHz.
            junk = psum.tile([128, CH], F32, tag="junk")
            warm_rhs = big.tile([128, CH], BF16, tag="warm_rhs")
            nc.vector.memset(warm_rhs[:, 0:CH], 0.0)
            for _ in range(10):
                nc.tensor.matmul(junk, ones128, warm_rhs, start=True, stop=True)

            # D''[i, n] = -2 a_i.f_n + ||f_n||^2   (anchor bias added in Ln)
            dbanks = []
            for j in range(NCH):
                dbank = psum.tile([128, CH], F32, tag=f"dbank{j}")
                dbanks.append(dbank)
            for j in range(NCH):
                sl = slice(j * CH, (j + 1) * CH)
                nc.tensor.matmul(
                    dbanks[j], atm2, ft[:, sl], start=True, stop=False
                )
                nc.tensor.matmul(dbanks[j], ones128, ft2[:, sl], start=False, stop=True)

            # Diagonals of D' (PSUM bank 0): self term D'[i,i] (cols 0:128)
            # and positive term D'[i, 128+i] (cols 128:256).
            scr0 = small.tile([128, 128], F32, tag="scr0")
            scr1 = small.tile([128, 128], F32, tag="scr1")
            selfd = small.tile([128, 1], F32, tag="selfd")
            posd = small.tile([128, 1], F32, tag="posd")
            nc.vector.tensor_mul(scr0, dbanks[0][:, 0:128], ident)
            nc.vector.tensor_reduce(
                selfd, scr0, axis=mybir.AxisListType.X, op=mybir.AluOpType.add
            )
            nc.vector.tensor_mul(scr1, dbanks[0][:, 128:256], ident)
            nc.vector.tensor_reduce(
                posd, scr1, axis=mybir.AxisListType.X, op=mybir.AluOpType.add
            )
            # D'' diagonals still need the per-anchor (1 + ||a_i||^2) term
            nc.vector.tensor_tensor(selfd, selfd, asq1_col, mybir.AluOpType.add)
            nc.vector.tensor_tensor(posd, posd, asq1_col, mybir.AluOpType.add)

            # P = exp(-ln(D')) on ACT (custom-DVE recip doesn't compile on
            # this walrus build; Ln+Exp share one ACT table set). The Exp
            # pass emits the row-sum for free via accum_out.
            lnd = big.tile([128, N], F32, tag="lnd")
            s_parts = []
            for j in range(NCH):
                sl = slice(j * CH, (j + 1) * CH)
                nc.scalar.activation(
                    lnd[:, sl],
                    dbanks[j],
                    mybir.ActivationFunctionType.Ln,
                    bias=asq1_col,
                )
                nc.scalar.activation(
                    prob[:, sl],
                    lnd[:, sl],
                    mybir.ActivationFunctionType.Exp,
                    scale=-1.0,
                )
                s_j = small.tile([128, 1], F32, tag=f"s_part{j}")
                nc.vector.tensor_reduce(
                    s_j, prob[:, sl], axis=mybir.AxisListType.X,
                    op=mybir.AluOpType.add,
                )
                s_parts.append(s_j)

            s01 = small.tile([128, 1], F32, tag="s01")
            s23 = small.tile([128, 1], F32, tag="s23")
            s_all = small.tile([128, 1], F32, tag="s_all")
            nc.vector.tensor_tensor(s01, s_parts[0], s_parts[1], mybir.AluOpType.add)
            nc.vector.tensor_tensor(s23, s_parts[2], s_parts[3], mybir.AluOpType.add)
            nc.vector.tensor_tensor(s_all, s01, s23, mybir.AluOpType.add)

            # S_i = rowsum - 1/D'[i,i];  loss_i = ln(S_i) + ln(D'[i,128+i])
            pself = small.tile([128, 1], F32, tag="pself")
            nc.vector.reciprocal(out=pself, in_=selfd)
            snet = small.tile([128, 1], F32, tag="snet")
            nc.vector.tensor_tensor(snet, s_all, pself, mybir.AluOpType.subtract)
            lgs = small.tile([128, 1], F32, tag="lgs")
            nc.scalar.activation(lgs, snet, mybir.ActivationFunctionType.Ln)
            lgp = small.tile([128, 1], F32, tag="lgp")
            nc.scalar.activation(lgp, posd, mybir.ActivationFunctionType.Ln)
            lossc = small.tile([128, 1], F32, tag="lossc")
            nc.vector.tensor_tensor(lossc, lgs, lgp, mybir.AluOpType.add)

            # Partial sum over this core's anchors -> [1, 1]
            psum_l = psum.tile([1, 1], F32, tag="psum_l")
            nc.tensor.matmul(psum_l, lossc, ones_col, start=True, stop=True)
            lsum = small.tile([1, 1], F32, tag="lsum")
            nc.scalar.copy(lsum, psum_l)
            nc.sync.dma_start(out=out[:, :], in_=lsum)

    return nc


_NC = None


def _canonical_inds():
    idx = np.arange(B)
    not_self = ~np.eye(B, dtype=bool)
    neg1 = np.broadcast_to(idx[None, :], (B, B))[not_self].reshape(B, B - 1)
    neg2 = neg1 + B
    pos = (idx + B)[:, None]
    return np.concatenate([pos, neg1, neg2], axis=1)


_CANON = None


def _is_canonical(neigh_inds):
    global _CANON
    if neigh_inds.shape != (B, 2 * B - 1):
        return False
    if _CANON is None:
        _CANON = _canonical_inds()
    return np.array_equal(np.asarray(neigh_inds, dtype=np.int64), _CANON)


def _run_fast(feats):
    global _NC, LAST_RESULT
    if _NC is None:
        _NC = _build_canonical()
    in_maps = []
    for c in range(NCORES):
        order = [c, NCORES + c] + [
            blk for blk in range(16) if blk not in (c, NCORES + c)
        ]
        rows = np.concatenate([np.arange(blk * 128, (blk + 1) * 128) for blk in order])
        ftp = np.ascontiguousarray(feats[rows].T)
        anch = np.ascontiguousarray(feats[c * 128 : (c + 1) * 128])
        in_maps.append({"ftp": ftp, "anch": anch})
    res = run_bass_kernel_spmd(_NC, in_maps, list(range(NCORES)), trace=TRACE)
    LAST_RESULT = res
    total = sum(float(res.results[c]["out"][0, 0]) for c in range(NCORES))
    return np.asarray(total / B, dtype=np.float32)


def _run_general(feats, neigh_inds):
    """Correctness fallback for non-canonical neighbor indices."""
    b = feats.shape[0] // 2
    origs = feats[:b]
    gram = origs @ feats.T
    sq = np.sum(feats * feats, axis=1)
    dists = sq[:b, None] + sq[None, :] - 2.0 * gram
    probs = 1.0 / (1.0 + dists)
    rows = np.arange(b)[:, None]
    sel = probs[rows, np.asarray(neigh_inds, dtype=np.int64)]
    loss = -(np.log(sel[:, 0]) - np.log(np.sum(sel, axis=1)))
    return np.asarray(np.mean(loss), dtype=np.float32)


def kernel(features, neigh_inds):
    feats = np.ascontiguousarray(np.asarray(features, dtype=np.float32))
    ni = np.asarray(neigh_inds)
    if _is_canonical(ni):
        return _run_fast(feats)
    return _run_general(feats, ni)

